# revision 5
# baseline (speedup 1.0000x reference)
"""Trainium2 Bass kernel for the BANLayer problem.

Computation (per batch):
    Uc   = relu(h_c @ U_w.T + U_b)            # (N, D)
    Vp   = relu(h_p @ V_w.T + V_b)            # (M, D)
    attn = Uc @ Vp.T                          # (N, M)
    w    = softmax(attn, axis=-1)
    ctx  = w @ Vp                             # (N, D)
    out  = mean_n((Uc + ctx) * q)             # (D,)

Device algorithm (data-parallel over batch, 8 cores x 8 batches):
    mean-factorized:  out = (q/N) * (sum_n Uc[n,:] + sum_m c[m] * Vp[m,:])
    with  c[m] = sum_n E[n,m] / s[n],  E = exp(attn - 30),  s[n] = sum_m E[n,m].
    The constant shift 30 is exact for softmax (shift invariance).

    All tensors enter the PE with the contraction dim on partitions, so the
    host pre-transposes h_c / h_p / weights (layout choice, free on host).
    s[n] comes for free from the exp pass via the ScalarE fused accumulator;
    c[m] is computed by one extra PE pass over E with a replicated-r
    stationary operand (gives c broadcast across all 128 partitions), and the
    final contraction sum_m c[m]*Vp[m,:] is a fused multiply-accumulate on
    the Vector engine.
"""

import sys

import numpy as np

sys.path.insert(0, "/opt/trn_rl_repo")

B, N, M, D = 64, 512, 1024, 128
CORES = 8
BL = B // CORES  # local batches per core
SHIFT = 30.0  # softmax logit shift (exact by shift invariance)

_BUILt = {}


def _build_nc():
    import concourse.bass as bass  # noqa: F401
    import concourse.tile as tile
    from concourse import bacc, mybir

    F32 = mybir.dt.float32
    F32R = mybir.dt.float32r
    OP = mybir.AluOpType
    ACTF = mybir.ActivationFunctionType

    nc = bacc.Bacc("TRN2", target_bir_lowering=False, debug=False,
                   num_devices=CORES)

    hcT = nc.declare_dram_parameter("hcT", [BL, D, N], F32, isOutput=False)
    hpT = nc.declare_dram_parameter("hpT", [BL, D, M], F32, isOutput=False)
    uwT = nc.declare_dram_parameter("uwT", [D, D], F32, isOutput=False)
    vwT = nc.declare_dram_parameter("vwT", [D, D], F32, isOutput=False)
    ub = nc.declare_dram_parameter("ub", [D, 1], F32, isOutput=False)
    vb = nc.declare_dram_parameter("vb", [D, 1], F32, isOutput=False)
    qn = nc.declare_dram_parameter("qn", [D, 1], F32, isOutput=False)
    y = nc.declare_dram_parameter("y", [D, BL], F32, isOutput=True)

    NCH = N // 128  # n-chunks per batch
    MH = M // 512   # m-halves per matmul free dim

    with tile.TileContext(nc) as tc:
        with (
            tc.tile_pool(name="consts", bufs=1) as consts,
            tc.tile_pool(name="inp", bufs=3) as inp,
            tc.tile_pool(name="proj", bufs=2) as proj,
            tc.tile_pool(name="epool", bufs=3) as epool,
            tc.tile_pool(name="scratch", bufs=2) as scratch,
            tc.tile_pool(name="stats", bufs=3) as stats,
            tc.tile_pool(name="psA", bufs=2, space="PSUM") as psA,
            tc.tile_pool(name="psB", bufs=1, space="PSUM") as psB,
            tc.tile_pool(name="psC", bufs=1, space="PSUM") as psC,
        ):
            uwT_sb = consts.tile([D, D], F32)
            nc.sync.dma_start(uwT_sb[:].bitcast(F32R), uwT[:].bitcast(F32R))
            vwT_sb = consts.tile([D, D], F32)
            nc.sync.dma_start(vwT_sb[:].bitcast(F32R), vwT[:].bitcast(F32R))
            ub_sb = consts.tile([D, 1], F32)
            nc.sync.dma_start(ub_sb[:], ub[:])
            vb_sb = consts.tile([D, 1], F32)
            nc.sync.dma_start(vb_sb[:], vb[:])
            qn_sb = consts.tile([D, 1], F32)
            nc.sync.dma_start(qn_sb[:], qn[:])
            zeros = consts.tile([128, 128], F32)
            nc.vector.memset(zeros[:], 0.0)
            nshift = consts.tile([128, 1], F32)
            nc.vector.memset(nshift[:], -SHIFT)
            y_sb = consts.tile([D, BL], F32)

            for b in range(BL):
                hc = inp.tile([D, N], F32, name="hc")
                nc.sync.dma_start(hc[:].bitcast(F32R), hcT[b].bitcast(F32R))
                hp = inp.tile([D, M], F32, name="hp")
                nc.sync.dma_start(hp[:].bitcast(F32R), hpT[b].bitcast(F32R))

                # UcT[e, n] = relu(U_w @ h_c[b].T + U_b); ucsum[e] = sum_n UcT
                uc_ps = psA.tile([128, 1024], F32, name="uc_ps", tag="att")
                nc.tensor.matmul(uc_ps[:, :N], uwT_sb[:].bitcast(F32R),
                                 hc[:].bitcast(F32R), start=True, stop=True)
                ucT = proj.tile([D, N], F32, name="ucT")
                ucsum = stats.tile([D, 1], F32, name="ucsum")
                nc.vector.tensor_scalar(ucT[:].bitcast(F32R), uc_ps[:, :N],
                                        ub_sb[:], 0.0, OP.add, OP.max)
                nc.vector.tensor_reduce(ucsum[:], ucT[:],
                                        mybir.AxisListType.X, OP.add)

                # VpT[e, m] = relu(V_w @ h_p[b].T + V_b)
                vp_ps = psB.tile([128, M], F32, name="vp_ps")
                for h in range(MH):
                    nc.tensor.matmul(vp_ps[:, h * 512:(h + 1) * 512],
                                     vwT_sb[:].bitcast(F32R),
                                     hp[:, h * 512:(h + 1) * 512].bitcast(F32R),
                                     start=True, stop=True)
                vpT = proj.tile([D, M], F32, name="vpT")
                nc.vector.tensor_scalar(vpT[:].bitcast(F32R), vp_ps[:],
                                        vb_sb[:], 0.0, OP.add, OP.max)

                cb_ps = psC.tile([128, M], F32, name="cb_ps")
                s4 = stats.tile([128, NCH], F32, name="s4")
                r4 = stats.tile([128, NCH], F32, name="r4")

                for j in range(NCH):
                    # attn chunk: [n=128, m=1024]
                    att_ps = psA.tile([128, 1024], F32, name="att_ps",
                                      tag="att")
                    lhs = ucT[:, j * 128:(j + 1) * 128]
                    for h in range(MH):
                        nc.tensor.matmul(att_ps[:, h * 512:(h + 1) * 512],
                                         lhs.bitcast(F32R),
                                         vpT[:, h * 512:(h + 1) * 512]
                                         .bitcast(F32R),
                                         start=True, stop=True)
                    # E = exp(attn - SHIFT) with fused row sums s[n]
                    e_sb = epool.tile([128, M], F32, name="e_sb")
                    nc.scalar.activation(e_sb[:].bitcast(F32R), att_ps[:],
                                         ACTF.Exp, bias=nshift[:], scale=1.0,
                                         accum_out=s4[:, j:j + 1])
                    # r = 1/s ; replicate along free dim for the PE pass
                    nc.vector.reciprocal(r4[:, j:j + 1], s4[:, j:j + 1])
                    r_rep = stats.tile([128, 128], F32, name="r_rep")
                    nc.vector.tensor_scalar(r_rep[:].bitcast(F32R), zeros[:],
                                            r4[:, j:j + 1], None, OP.add)
                    # c[m] broadcast over partitions: cb[p, m] += sum_n r[n]E[n, m]
                    for h in range(MH):
                        nc.tensor.matmul(cb_ps[:, h * 512:(h + 1) * 512],
                                         r_rep[:].bitcast(F32R),
                                         e_sb[:, h * 512:(h + 1) * 512]
                                         .bitcast(F32R),
                                         start=(j == 0), stop=(j == NCH - 1))

                # Yctx[e] = sum_m VpT[e,m] * c[m]
                dump = scratch.tile([128, M], F32, name="dump")
                yctx = stats.tile([D, 1], F32, name="yctx")
                nc.vector.scalar_tensor_tensor(dump[:], vpT[:], 1.0, cb_ps[:],
                                               OP.mult, OP.mult,
                                               accum_out=yctx[:])
                # y[:, b] = (ucsum + yctx) * q/N
                tsum = stats.tile([D, 1], F32, name="tsum")
                nc.vector.tensor_tensor(tsum[:], ucsum[:], yctx[:], OP.add)
                nc.vector.tensor_scalar(y_sb[:, b:b + 1], tsum[:], qn_sb[:],
                                        None, OP.mult)

            nc.sync.dma_start(y[:], y_sb[:])

    nc.finalize()
    return nc


def kernel(h_c, h_p, U_w, U_b, V_w, V_b, q):
    from concourse.bass_utils import run_bass_kernel_spmd

    if "nc" not in _BUILt:
        _BUILt["nc"] = _build_nc()
    nc = _BUILt["nc"]

    h_c = np.ascontiguousarray(np.asarray(h_c, dtype=np.float32))
    h_p = np.ascontiguousarray(np.asarray(h_p, dtype=np.float32))
    uwT = np.ascontiguousarray(np.asarray(U_w, dtype=np.float32).T)
    vwT = np.ascontiguousarray(np.asarray(V_w, dtype=np.float32).T)
    ub = np.asarray(U_b, dtype=np.float32).reshape(D, 1)
    vb = np.asarray(V_b, dtype=np.float32).reshape(D, 1)
    qn = (np.asarray(q, dtype=np.float32) / np.float32(N)).reshape(D, 1)

    in_maps = []
    for c in range(CORES):
        sl = slice(c * BL, (c + 1) * BL)
        in_maps.append({
            "hcT": np.ascontiguousarray(h_c[sl].transpose(0, 2, 1)),
            "hpT": np.ascontiguousarray(h_p[sl].transpose(0, 2, 1)),
            "uwT": uwT, "vwT": vwT, "ub": ub, "vb": vb, "qn": qn,
        })

    global _last_in_maps
    _last_in_maps = in_maps
    res = run_bass_kernel_spmd(nc, in_maps, core_ids=list(range(CORES)))
    out = np.empty((B, D), dtype=np.float32)
    for c in range(CORES):
        out[c * BL:(c + 1) * BL] = res.results[c]["y"].T
    return out


# revision 29
# speedup vs baseline: 1149.8262x; 1149.8262x over previous
"""Trainium2 Bass kernel for the BANLayer problem.

Computation (per batch):
    Uc   = relu(h_c @ U_w.T + U_b)            # (N, D)
    Vp   = relu(h_p @ V_w.T + V_b)            # (M, D)
    attn = Uc @ Vp.T                          # (N, M)
    w    = softmax(attn, axis=-1)
    ctx  = w @ Vp                             # (N, D)
    out  = mean_n((Uc + ctx) * q)             # (D,)

Device algorithm (data-parallel over batch, 8 cores x 8 batches):
    mean-factorized:  out = (q/N) * (sum_n Uc[n,:] + sum_m c[m] * Vp[m,:])
    with  c[m] = sum_n E[n,m] / s[n],  E = exp(attn - 30),  s[n] = sum_m E[n,m].
    The constant shift 30 is exact for softmax (shift invariance).

    All tensors enter the PE with the contraction dim on partitions, so the
    host pre-transposes h_c / h_p / weights (layout choice, free on host).
    s[n] comes for free from the exp pass via the ScalarE fused accumulator;
    c[m] is computed by one extra PE pass over E with a replicated-r
    stationary operand (gives c broadcast across all 128 partitions), and the
    final contraction sum_m c[m]*Vp[m,:] is a fused multiply-accumulate on
    the Vector engine.
"""

import sys

import numpy as np

sys.path.insert(0, "/opt/trn_rl_repo")

B, N, M, D = 64, 512, 1024, 128
CORES = 8
BL = B // CORES  # local batches per core
SHIFT = 40.0  # softmax logit shift (exact by shift invariance); keeps
              # exp() far from fp32 overflow for attn logits up to ~130

_BUILt = {}


def _build_nc():
    import concourse.bass as bass  # noqa: F401
    import concourse.tile as tile
    from concourse import bacc, mybir

    F32 = mybir.dt.float32
    F32R = mybir.dt.float32r
    OP = mybir.AluOpType
    ACTF = mybir.ActivationFunctionType

    nc = bacc.Bacc("TRN2", target_bir_lowering=False, debug=False,
                   num_devices=CORES)

    hcT = nc.declare_dram_parameter("hcT", [BL, D, N], F32, isOutput=False)
    hpT = nc.declare_dram_parameter("hpT", [BL, D, M], F32, isOutput=False)
    w2 = nc.declare_dram_parameter("w2", [D, 2 * D], F32, isOutput=False)
    bias3 = nc.declare_dram_parameter("bias3", [D, 3], F32, isOutput=False)
    y = nc.declare_dram_parameter("y", [D, BL], F32, isOutput=True)

    NCH = N // 128  # n-chunks per batch
    MH = M // 512   # m-halves per matmul free dim

    with tile.TileContext(nc) as tc:
        with (
            tc.tile_pool(name="consts", bufs=1) as consts,
            tc.tile_pool(name="inp", bufs=3) as inp,
            tc.tile_pool(name="proj", bufs=3) as proj,
            tc.tile_pool(name="epool", bufs=4) as epool,
            tc.tile_pool(name="scratch", bufs=2) as scratch,
            tc.tile_pool(name="stats", bufs=4) as stats,
            tc.tile_pool(name="psA", bufs=2, space="PSUM") as psA,
            tc.tile_pool(name="psB", bufs=1, space="PSUM") as psB,
            tc.tile_pool(name="psC", bufs=1, space="PSUM") as psC,
        ):
            # consts first on each dispatch queue (they gate the first MMs),
            # then batch inputs in order. hc loads dispatch on SP (HWDGE),
            # hp loads on Pool (SWDGE) so the two queues run in parallel.
            w2_sb = consts.tile([D, 2 * D], F32)
            nc.sync.dma_start(w2_sb[:].bitcast(F32R), w2[:].bitcast(F32R))
            uwT_sb = w2_sb[:, 0:D]
            vwT_sb = w2_sb[:, D:2 * D]
            b3_sb = consts.tile([D, 3], F32)
            nc.gpsimd.dma_start(b3_sb[:], bias3[:])
            ub_sb = b3_sb[:, 0:1]
            vb_sb = b3_sb[:, 1:2]
            qn_sb = b3_sb[:, 2:3]

            hcs, hps = [], []

            def load_batch(b):
                hc = inp.tile([D, N], F32, name="hc")
                hp = inp.tile([D, M], F32, name="hp")
                nc.sync.dma_start(hc[:].bitcast(F32R), hcT[b].bitcast(F32R))
                nc.gpsimd.dma_start(hp[:].bitcast(F32R), hpT[b].bitcast(F32R))
                hcs.append(hc)
                hps.append(hp)

            load_batch(0)
            zeros = consts.tile([128, 128], F32)
            nc.vector.memset(zeros[:], 0.0)
            nshift = consts.tile([128, 1], F32)
            nc.vector.memset(nshift[:], -SHIFT)
            y_sb = consts.tile([D, BL], F32)

            # pre-touch the consts on PE / DVE: fp32 matmuls carry at most one
            # semaphore wait, so observing w2/b3 here keeps the hot-path ops
            # single-wait (no event-semaphore indirection on the head chain)
            wu_ps = psA.tile([1, 1], F32, name="wu_ps", tag="att")
            nc.tensor.matmul(wu_ps[:], w2_sb[:, 0:1], w2_sb[:, 0:1],
                             start=True, stop=True)
            wu_sb = stats.tile([128, 1], F32, name="wu_sb")
            nc.vector.tensor_scalar(wu_sb[:], b3_sb[:, 0:1], 0.0, None,
                                    OP.mult)

            for b in range(BL):
                hc = hcs[b]
                hp = hps[b]

                # UcT[e, n] = relu(U_w @ h_c[b].T + U_b); ucsum[e] = sum_n UcT
                uc_ps = psB.tile([128, N], F32, name="uc_ps", tag="proj", bufs=2)
                nc.tensor.matmul(uc_ps[:], uwT_sb.bitcast(F32R),
                                 hc[:].bitcast(F32R), start=True, stop=True)
                ucT = proj.tile([D, N], F32, name="ucT")
                ucsum = stats.tile([D, 1], F32, name="ucsum")
                nc.vector.tensor_scalar(ucT[:].bitcast(F32R), uc_ps[:],
                                        ub_sb, 0.0, OP.add, OP.max)
                nc.vector.tensor_reduce(ucsum[:], ucT[:],
                                        mybir.AxisListType.X, OP.add)

                # VpT[e, m] = relu(V_w @ h_p[b].T + V_b); one PSUM bank,
                # halves evicted between matmuls
                vpT = proj.tile([D, M], F32, name="vpT")
                for h in range(MH):
                    vp_ps = psB.tile([128, 512], F32, name="vp_ps", tag="proj", bufs=2)
                    nc.tensor.matmul(vp_ps[:],
                                     vwT_sb.bitcast(F32R),
                                     hp[:, h * 512:(h + 1) * 512].bitcast(F32R),
                                     start=True, stop=True)
                    nc.vector.tensor_scalar(
                        vpT[:, h * 512:(h + 1) * 512].bitcast(F32R),
                        vp_ps[:],
                        vb_sb, 0.0, OP.add, OP.max)

                cb_ps = psC.tile([128, M], F32, name="cb_ps")
                s4 = stats.tile([128, NCH], F32, name="s4")
                r4 = stats.tile([128, NCH], F32, name="r4")

                while len(hcs) < min(b + 3, BL):
                    load_batch(len(hcs))

                for j in range(NCH):
                    # attn chunk: [n=128, m=1024]
                    att_ps = psA.tile([128, 1024], F32, name="att_ps",
                                      tag="att")
                    lhs = ucT[:, j * 128:(j + 1) * 128]
                    for h in range(MH):
                        nc.tensor.matmul(att_ps[:, h * 512:(h + 1) * 512],
                                         lhs.bitcast(F32R),
                                         vpT[:, h * 512:(h + 1) * 512]
                                         .bitcast(F32R),
                                         start=True, stop=True)
                    # E = exp(attn - SHIFT) with fused row sums s[n]
                    e_sb = epool.tile([128, M], F32, name="e_sb")
                    nc.scalar.activation(e_sb[:].bitcast(F32R), att_ps[:],
                                         ACTF.Exp, bias=nshift[:], scale=1.0,
                                         accum_out=s4[:, j:j + 1])
                    # r = 1/s ; replicate along free dim for the PE pass
                    nc.vector.reciprocal(r4[:, j:j + 1], s4[:, j:j + 1])
                    r_rep = stats.tile([128, 128], F32, name="r_rep")
                    nc.vector.tensor_scalar(r_rep[:].bitcast(F32R), zeros[:],
                                            r4[:, j:j + 1], None, OP.add)
                    # c[m] broadcast over partitions: cb[p, m] += sum_n r[n]E[n, m]
                    for h in range(MH):
                        nc.tensor.matmul(cb_ps[:, h * 512:(h + 1) * 512],
                                         r_rep[:].bitcast(F32R),
                                         e_sb[:, h * 512:(h + 1) * 512]
                                         .bitcast(F32R),
                                         start=(j == 0), stop=(j == NCH - 1))

                # Yctx[e] = sum_m VpT[e,m] * c[m]
                dump = scratch.tile([128, M], F32, name="dump")
                yctx = stats.tile([D, 1], F32, name="yctx")
                nc.vector.scalar_tensor_tensor(dump[:], vpT[:], 1.0, cb_ps[:],
                                               OP.mult, OP.mult,
                                               accum_out=yctx[:])
                # y[:, b] = (ucsum + yctx) * q/N
                tsum = stats.tile([D, 1], F32, name="tsum")
                nc.gpsimd.tensor_tensor(tsum[:], ucsum[:], yctx[:], OP.add)
                nc.gpsimd.tensor_scalar(y_sb[:, b:b + 1], tsum[:], qn_sb,
                                        None, OP.mult)

            nc.sync.dma_start(y[:], y_sb[:])

    nc.finalize()
    return nc


def kernel(h_c, h_p, U_w, U_b, V_w, V_b, q):
    from concourse.bass_utils import run_bass_kernel_spmd

    if "nc" not in _BUILt:
        _BUILt["nc"] = _build_nc()
    nc = _BUILt["nc"]

    h_c = np.ascontiguousarray(np.asarray(h_c, dtype=np.float32))
    h_p = np.ascontiguousarray(np.asarray(h_p, dtype=np.float32))
    w2 = np.ascontiguousarray(np.concatenate(
        [np.asarray(U_w, dtype=np.float32).T,
         np.asarray(V_w, dtype=np.float32).T], axis=1))
    bias3 = np.ascontiguousarray(np.stack(
        [np.asarray(U_b, dtype=np.float32),
         np.asarray(V_b, dtype=np.float32),
         np.asarray(q, dtype=np.float32) / np.float32(N)], axis=1))

    in_maps = []
    for c in range(CORES):
        sl = slice(c * BL, (c + 1) * BL)
        in_maps.append({
            "hcT": np.ascontiguousarray(h_c[sl].transpose(0, 2, 1)),
            "hpT": np.ascontiguousarray(h_p[sl].transpose(0, 2, 1)),
            "w2": w2, "bias3": bias3,
        })

    global _last_in_maps
    _last_in_maps = in_maps
    res = run_bass_kernel_spmd(nc, in_maps, core_ids=list(range(CORES)))
    out = np.empty((B, D), dtype=np.float32)
    for c in range(CORES):
        out[c * BL:(c + 1) * BL] = res.results[c]["y"].T
    return out
